# revision 1
# baseline (speedup 1.0000x reference)
"""Deformable conv (nn_DeformConv) Trainium2 Bass kernel.

Strategy (per core = one batch of 8, data-parallel):
  1. 1x1 conv (PE) + depthwise 3x3 (DVE, shifted views) -> offsets [18, 4096]
  2. PE-transpose offsets to position-partition layout; batched per-position
     floor/residual math and flat gather index r0 into a zero-padded 72x72 grid.
  3. DRAM table [5248 rows, 1024] bf16, row r = [x[r] | Dx[r] | Dy[r] | Dxy[r]]
     (finite differences of zero-padded x). Bilinear sample ==
     x[r0] + rx*Dx[r0] + ry*Dy[r0] + rx*ry*Dxy[r0] (exact, incl. OOB zeroing).
  4. Per 128-position tile: indirect row-gathers (per tap), wide broadcast-AP
     multiply + 3 adds for the combine, PE-transpose sampled into 18 ck-tiles,
     PSUM-accumulated matmul against w_def (bf16), DMA out.
"""
import os
import numpy as np
from contextlib import ExitStack

import concourse.bass as bass
import concourse.mybir as mybir
import concourse.tile as tile
from concourse import bacc as _bacc
from concourse.bass import IndirectOffsetOnAxis
from concourse.masks import make_identity

FP32 = mybir.dt.float32
BF16 = mybir.dt.bfloat16
I32 = mybir.dt.int32

N, C, H, W = 8, 256, 64, 64
HW = H * W                    # 4096
K = 9
OFFC = 18
PAD = 4
G = H + 2 * PAD               # 72
ROWS = G * G                  # 5184
RT = 5248                     # rows padded to 41*128
NRT = RT // 128               # 41
NPT = HW // 128               # 32 position tiles
CT = C // 128                 # 2 channel tiles
KT = (C * K) // 128           # 18 contraction tiles
ALU = mybir.AluOpType
AF = mybir.ActivationFunctionType

MODE = os.environ.get("MODE", "full")   # full | pre
# NOTE: multi-offset indirect DMA fails at runtime on HW; keep per-tap gathers.
MERGE_GATHER = os.environ.get("MERGE_GATHER", "0") == "1"


def build_nc():
    nc = _bacc.Bacc()
    x_d = nc.dram_tensor("x", [C, HW], FP32, kind="ExternalInput")
    w_adj_d = nc.dram_tensor("w_adj", [OFFC, C], FP32, kind="ExternalInput")
    b_adj_d = nc.dram_tensor("b_adj", [OFFC, 1], FP32, kind="ExternalInput")
    w_off_d = nc.dram_tensor("w_off", [OFFC, K], FP32, kind="ExternalInput")
    b_off_d = nc.dram_tensor("b_off", [OFFC, 1], FP32, kind="ExternalInput")
    w_def_d = nc.dram_tensor("w_def", [C, C * K], FP32, kind="ExternalInput")
    out_d = nc.dram_tensor("out", [C, HW], FP32, kind="ExternalOutput")

    with tile.TileContext(nc) as tc, ExitStack() as ctx:
        pers = ctx.enter_context(tc.tile_pool(name="pers", bufs=1))
        dram = ctx.enter_context(tc.tile_pool(name="dram", bufs=1, space="DRAM"))

        table = dram.tile([RT, 4 * C], BF16)

        ident_f = pers.tile([128, 128], FP32)
        make_identity(nc, ident_f[:])
        ident_b = pers.tile([128, 128], BF16)
        nc.vector.tensor_copy(ident_b[:], ident_f[:])

        # per-partition constants: hh = p//64 (0/1), ww = p%64
        iota_p = pers.tile([128, 1], I32)
        nc.gpsimd.iota(iota_p[:], pattern=[[0, 1]], base=0, channel_multiplier=1)
        pf = pers.tile([128, 1], FP32)
        nc.vector.tensor_copy(pf[:], iota_p[:])
        hh = pers.tile([128, 1], FP32)
        nc.vector.tensor_scalar(out=hh[:], in0=pf[:], scalar1=64.0, scalar2=None,
                                op0=ALU.is_ge)
        ww = pers.tile([128, 1], FP32)
        nc.vector.scalar_tensor_tensor(out=ww[:], in0=hh[:], scalar=-64.0,
                                       in1=pf[:], op0=ALU.mult, op1=ALU.add)

        # batched base ramps over (t, k): by = 2t + ki + (PAD-1), bx = kj + (PAD-1)
        by_i = pers.tile([128, NPT, K], I32)
        nc.gpsimd.iota(by_i[:], pattern=[[2, NPT], [1, 3], [0, 3]], base=PAD - 1,
                       channel_multiplier=0)
        bx_i = pers.tile([128, NPT, K], I32)
        nc.gpsimd.iota(bx_i[:], pattern=[[0, NPT], [0, 3], [1, 3]], base=PAD - 1,
                       channel_multiplier=0)
        by_f = pers.tile([128, NPT, K], FP32)
        nc.vector.tensor_copy(by_f[:], by_i[:])
        bx_f = pers.tile([128, NPT, K], FP32)
        nc.vector.tensor_copy(bx_f[:], bx_i[:])

        w_defT = pers.tile([128, KT, 2 * 128], BF16)   # [ck-part, kt, o]
        r0_sb = pers.tile([128, NPT, K], I32)
        wts_sb = pers.tile([128, NPT, K * 3], FP32)    # k-major (rx, ry, rxry)

        # ---------------- phase 3: w_def transpose (overlaps phase 1) ----------------
        xs_stack = ExitStack()
        xp = xs_stack.enter_context(tc.tile_pool(name="xp", bufs=1))
        with tc.tile_pool(name="psW", bufs=4, space="PSUM") as psW:
            w_def_sb = xp.tile([128, 2, C * K], FP32)
            for ot in range(2):
                nc.sync.dma_start(out=w_def_sb[:, ot, :],
                                  in_=w_def_d[ot * 128:(ot + 1) * 128, :])
            for kt in range(KT):
                k = kt // 2
                chalf = kt % 2
                for ot in range(2):
                    ps = psW.tile([128, 128], FP32, tag="psw")
                    src = w_def_sb[:, ot, :].rearrange("p (c k) -> p k c", k=K) \
                        [:, k, chalf * 128:(chalf + 1) * 128]
                    nc.tensor.transpose(ps[:], src, ident_f[:])
                    nc.scalar.copy(w_defT[:, kt, ot * 128:ot * 128 + 128], ps[:])

        # ---------------- phase 1: offsets pipeline ----------------
        x_sb = xp.tile([128, CT, HW], FP32)
        for ct in range(CT):
            nc.sync.dma_start(out=x_sb[:, ct, :], in_=x_d[ct * 128:(ct + 1) * 128, :])

        ph1 = ExitStack()
        offp = ph1.enter_context(tc.tile_pool(name="offp", bufs=1))
        psA = ph1.enter_context(tc.tile_pool(name="psA", bufs=2, space="PSUM"))

        w_adjT = offp.tile([128, CT, OFFC], FP32)
        for ct in range(CT):
            nc.sync.dma_start(
                out=w_adjT[:, ct, :],
                in_=w_adj_d.rearrange("o c -> c o")[ct * 128:(ct + 1) * 128, :])
        b_adj_sb = offp.tile([OFFC, 1], FP32)
        nc.sync.dma_start(out=b_adj_sb[:], in_=b_adj_d[:, :])
        w_off_sb = offp.tile([OFFC, K], FP32)
        nc.sync.dma_start(out=w_off_sb[:], in_=w_off_d[:, :])
        b_off_sb = offp.tile([OFFC, 1], FP32)
        nc.sync.dma_start(out=b_off_sb[:], in_=b_off_d[:, :])

        # 1x1 conv -> x_chan (padded 66x66 for the depthwise conv)
        GC = H + 2   # 66
        xch_pad = offp.tile([OFFC, GC * GC], BF16)
        nc.scalar.memzero(xch_pad[:])
        xch_v = xch_pad[:].rearrange("p (h w) -> p h w", h=GC, w=GC)
        for pch in range(8):
            ps = psA.tile([OFFC, 512], FP32)
            for ct in range(CT):
                nc.tensor.matmul(out=ps[:], lhsT=w_adjT[:, ct, :],
                                 rhs=x_sb[:, ct, pch * 512:(pch + 1) * 512],
                                 start=(ct == 0), stop=(ct == CT - 1))
            nc.scalar.activation(
                out=xch_v[:, 1 + pch * 8:1 + pch * 8 + 8, 1:1 + W],
                in_=ps[:].rearrange("p (h w) -> p h w", h=8, w=W),
                func=AF.Identity, bias=b_adj_sb[:], scale=1.0)

        # depthwise 3x3 -> offsets [18, 4096] (DVE chain, bf16)
        off_sb = offp.tile([OFFC, HW], BF16)
        ova = off_sb[:].rearrange("p (h w) -> p h w", h=H, w=W)
        for tap in range(K):
            di, dj = tap // 3, tap % 3
            vin = xch_v[:, di:di + H, dj:dj + W]
            if tap == 0:
                nc.vector.tensor_scalar(
                    out=ova, in0=vin, scalar1=w_off_sb[:, 0:1],
                    scalar2=b_off_sb[:, 0:1], op0=ALU.mult, op1=ALU.add)
            else:
                nc.vector.scalar_tensor_tensor(
                    out=ova, in0=vin, scalar=w_off_sb[:, tap:tap + 1],
                    in1=ova, op0=ALU.mult, op1=ALU.add)

        # transpose offsets to position-partition layout (batched index math)
        with tc.tile_pool(name="psT", bufs=4, space="PSUM") as psT, \
             tc.tile_pool(name="scr", bufs=1) as scr:
            offT = scr.tile([128, NPT, OFFC], FP32)
            for t in range(NPT):
                pso = psT.tile([128, OFFC], BF16, tag="pst")
                nc.tensor.transpose(pso[:], off_sb[:, t * 128:(t + 1) * 128],
                                    ident_b[:OFFC, :OFFC])
                nc.scalar.copy(offT[:, t, :], pso[:])

            dyv = offT[:].rearrange("p t (k two) -> p t k two", two=2)[:, :, :, 0]
            dxv = offT[:].rearrange("p t (k two) -> p t k two", two=2)[:, :, :, 1]
            py = scr.tile([128, NPT, K], FP32)
            px = scr.tile([128, NPT, K], FP32)
            nc.vector.scalar_tensor_tensor(out=py[:], in0=dyv, scalar=hh[:, 0:1],
                                           in1=by_f[:], op0=ALU.add, op1=ALU.add)
            nc.vector.scalar_tensor_tensor(out=px[:], in0=dxv, scalar=ww[:, 0:1],
                                           in1=bx_f[:], op0=ALU.add, op1=ALU.add)
            fyi = scr.tile([128, NPT, K], I32)
            fxi = scr.tile([128, NPT, K], I32)
            nc.vector.tensor_copy(fyi[:], py[:])
            nc.vector.tensor_copy(fxi[:], px[:])
            fy = scr.tile([128, NPT, K], FP32)
            fx = scr.tile([128, NPT, K], FP32)
            nc.vector.tensor_copy(fy[:], fyi[:])
            nc.vector.tensor_copy(fx[:], fxi[:])
            m = scr.tile([128, NPT, K], FP32)
            nc.vector.tensor_tensor(out=m[:], in0=fy[:], in1=py[:], op=ALU.is_gt)
            nc.vector.tensor_sub(out=fy[:], in0=fy[:], in1=m[:])
            nc.vector.tensor_tensor(out=m[:], in0=fx[:], in1=px[:], op=ALU.is_gt)
            nc.vector.tensor_sub(out=fx[:], in0=fx[:], in1=m[:])
            # residuals, k-major slots (rx, ry, rxry)
            wv = wts_sb[:].rearrange("p t (k s) -> p t k s", s=3)
            nc.vector.tensor_sub(out=wv[:, :, :, 0], in0=px[:], in1=fx[:])
            nc.vector.tensor_sub(out=wv[:, :, :, 1], in0=py[:], in1=fy[:])
            nc.vector.tensor_tensor(out=wv[:, :, :, 2], in0=wv[:, :, :, 0],
                                    in1=wv[:, :, :, 1], op=ALU.mult)
            r0f = scr.tile([128, NPT, K], FP32)
            nc.vector.scalar_tensor_tensor(out=r0f[:], in0=fy[:], scalar=float(G),
                                           in1=fx[:], op0=ALU.mult, op1=ALU.add)
            nc.vector.tensor_scalar(out=r0f[:], in0=r0f[:], scalar1=0.0,
                                    scalar2=float(RT - G - 2), op0=ALU.max,
                                    op1=ALU.min)
            nc.vector.tensor_copy(r0_sb[:], r0f[:])
        ph1.close()

        # ---------------- phase 2: table build (all bf16) ----------------
        with tc.tile_pool(name="tblp", bufs=1) as tblp, \
             tc.tile_pool(name="psB", bufs=4, space="PSUM") as psB, \
             tc.tile_pool(name="evb", bufs=3) as evb:
            xbf = tblp.tile([128, CT, RT], BF16)
            nc.scalar.memzero(xbf[:])
            dbf = tblp.tile([128, CT, 3, RT], BF16)
            for ct in range(CT):
                nc.vector.tensor_copy(
                    xbf[:, ct, :ROWS].rearrange("p (h w) -> p h w", h=G, w=G)
                        [:, PAD:PAD + H, PAD:PAD + W],
                    x_sb[:, ct, :].rearrange("p (h w) -> p h w", h=H, w=W))
            for ct in range(CT):
                nc.vector.tensor_sub(out=dbf[:, ct, 0, 0:RT - 1],
                                     in0=xbf[:, ct, 1:RT], in1=xbf[:, ct, 0:RT - 1])
                nc.gpsimd.memset(dbf[:, ct, 0, RT - 1:RT], 0.0)
                nc.vector.tensor_sub(out=dbf[:, ct, 1, 0:RT - G],
                                     in0=xbf[:, ct, G:RT], in1=xbf[:, ct, 0:RT - G])
                nc.gpsimd.memset(dbf[:, ct, 1, RT - G:RT], 0.0)
                nc.vector.tensor_sub(out=dbf[:, ct, 2, 0:RT - G],
                                     in0=dbf[:, ct, 0, G:RT], in1=dbf[:, ct, 0, 0:RT - G])
                nc.gpsimd.memset(dbf[:, ct, 2, RT - G:RT], 0.0)

            for rt in range(NRT):
                tb = evb.tile([128, 4, C], BF16, tag="tb")
                for ct in range(CT):
                    ps = psB.tile([128, 4 * 128], BF16, tag="ps")
                    nc.tensor.transpose(ps[:, 0:128],
                                        xbf[:, ct, rt * 128:(rt + 1) * 128], ident_b[:])
                    for s in range(3):
                        nc.tensor.transpose(
                            ps[:, (s + 1) * 128:(s + 2) * 128],
                            dbf[:, ct, s, rt * 128:(rt + 1) * 128], ident_b[:])
                    # one grouped evac: psum [128, 512] -> tb strided slots
                    tbv = tb[:, :, ct * 128:(ct + 1) * 128]
                    psv = ps[:].rearrange("p (s c) -> p s c", s=4)
                    if (rt + ct) % 2 == 0:
                        nc.scalar.copy(tbv, psv)
                    else:
                        nc.vector.tensor_copy(tbv, psv)
                nc.sync.dma_start(out=table[rt * 128:(rt + 1) * 128, :], in_=tb[:])
        xs_stack.close()

        if MODE == "pre":
            with tc.tile_pool(name="zz", bufs=1) as zz:
                zt = zz.tile([128, HW], FP32)
                nc.vector.memset(zt[:], 0.0)
                for ot in range(2):
                    nc.sync.dma_start(out=out_d[ot * 128:(ot + 1) * 128, :], in_=zt[:])
            return nc

        # ---------------- phase 4: main loop ----------------
        outp = ctx.enter_context(tc.tile_pool(name="outp", bufs=1))
        out_sb = outp.tile([128, 2, HW], FP32)
        with tc.tile_pool(name="gat", bufs=int(os.environ.get("GBUFS", "3"))) as gat, \
             tc.tile_pool(name="smp", bufs=int(os.environ.get("SBUFS", "2"))) as smp, \
             tc.tile_pool(name="psS", bufs=4, space="PSUM") as psS, \
             tc.tile_pool(name="psO", bufs=2, space="PSUM") as psO:
            for t in range(NPT):
                g_sb = gat.tile([128, K, 4 * C], BF16, tag="g")
                if MERGE_GATHER:
                    nc.gpsimd.indirect_dma_start(
                        out=g_sb[:], out_offset=None,
                        in_=table[:, :],
                        in_offset=IndirectOffsetOnAxis(ap=r0_sb[:, t, :], axis=0))
                else:
                    for k in range(K):
                        nc.gpsimd.indirect_dma_start(
                            out=g_sb[:, k, :], out_offset=None,
                            in_=table[:, :],
                            in_offset=IndirectOffsetOnAxis(ap=r0_sb[:, t, k:k + 1], axis=0))
                samp = smp.tile([128, KT * 128], BF16, tag="s")
                for k in range(K):
                    av = samp[:, k * C:(k + 1) * C]
                    eng = nc.vector if k < int(os.environ.get('DVE_TAPS', '9')) else nc.gpsimd
                    eng.scalar_tensor_tensor(
                        out=av, in0=g_sb[:, k, C:2 * C],
                        scalar=wts_sb[:, t, 3 * k:3 * k + 1],
                        in1=g_sb[:, k, 0:C], op0=ALU.mult, op1=ALU.add)
                    eng.scalar_tensor_tensor(
                        out=av, in0=g_sb[:, k, 2 * C:3 * C],
                        scalar=wts_sb[:, t, 3 * k + 1:3 * k + 2],
                        in1=av, op0=ALU.mult, op1=ALU.add)
                    eng.scalar_tensor_tensor(
                        out=av, in0=g_sb[:, k, 3 * C:4 * C],
                        scalar=wts_sb[:, t, 3 * k + 2:3 * k + 3],
                        in1=av, op0=ALU.mult, op1=ALU.add)

                sampT = smp.tile([128, KT, 128], BF16, tag="st")
                for q in range(5):   # groups of 4 transposes -> one evac
                    n_in_g = 4 if q < 4 else 2
                    ps = psS.tile([128, 4 * 128], BF16, tag="pss")
                    for j in range(n_in_g):
                        kt = q * 4 + j
                        nc.tensor.transpose(ps[:, j * 128:(j + 1) * 128],
                                            samp[:, kt * 128:(kt + 1) * 128], ident_b[:])
                    nc.scalar.copy(sampT[:, q * 4:q * 4 + n_in_g, :],
                                   ps[:, :n_in_g * 128])
                for ot in range(2):
                    pso = psO.tile([128, 128], FP32, tag="po")
                    for kt in range(KT):
                        nc.tensor.matmul(out=pso[:],
                                         lhsT=w_defT[:, kt, ot * 128:(ot + 1) * 128],
                                         rhs=sampT[:, kt, :],
                                         start=(kt == 0), stop=(kt == KT - 1))
                    nc.scalar.copy(out_sb[:, ot, t * 128:(t + 1) * 128], pso[:])
            for ot in range(2):
                nc.sync.dma_start(out=out_d[ot * 128:(ot + 1) * 128, :],
                                  in_=out_sb[:, ot, :])
    return nc


_CACHE = {}


def _get_nc():
    if "nc" not in _CACHE:
        nc = build_nc()
        if not nc.is_finalized():
            nc.finalize()
        _CACHE["nc"] = nc
    return _CACHE["nc"]


def kernel(**inputs):
    from concourse import bass_utils
    x = np.ascontiguousarray(inputs["x"], dtype=np.float32)          # [8,256,64,64]
    w_adj = np.ascontiguousarray(inputs["w_adj"], dtype=np.float32).reshape(OFFC, C)
    b_adj = np.ascontiguousarray(inputs["b_adj"], dtype=np.float32).reshape(OFFC, 1)
    w_off = np.ascontiguousarray(inputs["w_off"], dtype=np.float32).reshape(OFFC, K)
    b_off = np.ascontiguousarray(inputs["b_off"], dtype=np.float32).reshape(OFFC, 1)
    w_def = np.ascontiguousarray(inputs["w_def"], dtype=np.float32).reshape(C, C * K)

    nc = _get_nc()
    in_maps = []
    for n in range(N):
        in_maps.append({
            "x": np.ascontiguousarray(x[n].reshape(C, HW)),
            "w_adj": w_adj, "b_adj": b_adj,
            "w_off": w_off, "b_off": b_off,
            "w_def": w_def,
        })
    res = bass_utils.run_bass_kernel_spmd(nc, in_maps, core_ids=list(range(N)))
    outs = [res.results[n]["out"].reshape(C, H, W) for n in range(N)]
    return np.stack(outs, axis=0)


if __name__ == "__main__":
    nc = build_nc()
    print("build ok")



# revision 3
# speedup vs baseline: 1.4873x; 1.4873x over previous
"""Deformable conv (nn_DeformConv) Trainium2 Bass kernel, v3.

Per core = one batch element, data-parallel across 8 cores.

Phases 1 (offsets), 2 (table build), 3 (w_def transpose) share one pool
scope so the tile scheduler can overlap them freely; all close before the
main loop.

Main loop per 128-position tile:
  - 9 per-tap indirect row gathers from the DRAM table
    [x | Dx | Dy | Dxy] (2KB bf16 rows)  [GATHER=tap]
    or 1 multi-idx gather + token + fixup-gather + fixup DMA [GATHER=multi].
  - combine+transpose fused on PE: per (tap, chalf) accumulate in PSUM
    T(x) + T(Dx)·diag(rx) + T(Dy)·diag(ry) + T(Dxy)·diag(rxry);
    diag mats built per tile by DVE tensor_scalar (4x perf mode).
  - PSUM-accumulated matmul against w_defT -> out, out DMA per 8 tiles.
"""
import os
import numpy as np
from contextlib import ExitStack

import concourse.bass as bass
import concourse.mybir as mybir
import concourse.tile as tile
from concourse import bacc as _bacc
from concourse.bass import IndirectOffsetOnAxis
from concourse.masks import make_identity

FP32 = mybir.dt.float32
BF16 = mybir.dt.bfloat16
I32 = mybir.dt.int32

N, C, H, W = 8, 256, 64, 64
HW = H * W                    # 4096
K = 9
OFFC = 18
PAD = 4
G = H + 2 * PAD               # 72
ROWS = G * G                  # 5184
RT = 5248                     # rows padded to 41*128
NRT = RT // 128               # 41
NPT = HW // 128               # 32 position tiles
CT = C // 128                 # 2 channel tiles
KT = (C * K) // 128           # 18 contraction tiles
ALU = mybir.AluOpType
AF = mybir.ActivationFunctionType

GATHER = os.environ.get("GATHER", "tap")   # tap | multi


def build_nc():
    nc = _bacc.Bacc()
    x_d = nc.dram_tensor("x", [C, HW], FP32, kind="ExternalInput")
    w_adj_d = nc.dram_tensor("w_adj", [OFFC, C], FP32, kind="ExternalInput")
    b_adj_d = nc.dram_tensor("b_adj", [OFFC, 1], FP32, kind="ExternalInput")
    w_off_d = nc.dram_tensor("w_off", [OFFC, K], FP32, kind="ExternalInput")
    b_off_d = nc.dram_tensor("b_off", [OFFC, 1], FP32, kind="ExternalInput")
    w_def_d = nc.dram_tensor("w_def", [C, C * K], FP32, kind="ExternalInput")
    out_d = nc.dram_tensor("out", [C, HW], FP32, kind="ExternalOutput")

    with tile.TileContext(nc) as tc, ExitStack() as ctx:
        pers = ctx.enter_context(tc.tile_pool(name="pers", bufs=1))
        dram = ctx.enter_context(tc.tile_pool(name="dram", bufs=1, space="DRAM"))

        BND = 2816               # band rows (22 chunks)
        HB = 2432                # hi-band start (chunk-aligned)
        table_lo = dram.tile([BND, 4 * C], BF16)
        table_hi = dram.tile([BND, 4 * C], BF16)

        ident_f = pers.tile([128, 128], FP32)
        make_identity(nc, ident_f[:])
        ident_b = pers.tile([128, 128], BF16)
        nc.vector.tensor_copy(ident_b[:], ident_f[:])

        # per-partition constants: hh = p//64 (0/1), ww = p%64
        iota_p = pers.tile([128, 1], I32)
        nc.gpsimd.iota(iota_p[:], pattern=[[0, 1]], base=0, channel_multiplier=1)
        pf = pers.tile([128, 1], FP32)
        nc.vector.tensor_copy(pf[:], iota_p[:])
        hh = pers.tile([128, 1], FP32)
        nc.vector.tensor_scalar(out=hh[:], in0=pf[:], scalar1=64.0, scalar2=None,
                                op0=ALU.is_ge)
        ww = pers.tile([128, 1], FP32)
        nc.vector.scalar_tensor_tensor(out=ww[:], in0=hh[:], scalar=-64.0,
                                       in1=pf[:], op0=ALU.mult, op1=ALU.add)

        # batched base ramps over (t, k): by = 2t + ki + (PAD-1), bx = kj + (PAD-1)
        by_i = pers.tile([128, NPT, K], I32)
        nc.gpsimd.iota(by_i[:], pattern=[[2, NPT], [1, 3], [0, 3]], base=PAD - 1,
                       channel_multiplier=0)
        bx_i = pers.tile([128, NPT, K], I32)
        nc.gpsimd.iota(bx_i[:], pattern=[[0, NPT], [0, 3], [1, 3]], base=PAD - 1,
                       channel_multiplier=0)
        by_f = pers.tile([128, NPT, K], FP32)
        nc.vector.tensor_copy(by_f[:], by_i[:])
        bx_f = pers.tile([128, NPT, K], FP32)
        nc.vector.tensor_copy(bx_f[:], bx_i[:])

        w_defT = pers.tile([128, KT, 2 * 128], BF16)   # [ck-part, kt, o]
        r0_sb = pers.tile([128, NPT, K], I32)
        wts_sb = pers.tile([128, NPT, K * 3], FP32)    # k-major (rx, ry, rxry)
        r0T_f = pers.tile([108, 3], FP32)              # fixup staging
        r0T2_f = pers.tile([9, 36], FP32)              # r0T2_f[k, t]

        # ---------------- phases 1+3 scope (freed before main loop);
        # phase-2 pools stay open so the hi-band build overlaps the loop ----
        tblp = ctx.enter_context(tc.tile_pool(name="tblp", bufs=1))
        evb = ctx.enter_context(tc.tile_pool(name="evb", bufs=3))
        dyb = ctx.enter_context(tc.tile_pool(name="dyb", bufs=3))
        psB = ctx.enter_context(tc.tile_pool(name="psB", bufs=3, space="PSUM"))
        phx = ExitStack()
        xp = phx.enter_context(tc.tile_pool(name="xp", bufs=1))
        offp = phx.enter_context(tc.tile_pool(name="offp", bufs=1))
        scr = phx.enter_context(tc.tile_pool(name="scr", bufs=1))
        psW = phx.enter_context(tc.tile_pool(name="psW", bufs=1, space="PSUM"))
        psA = phx.enter_context(tc.tile_pool(name="psA", bufs=1, space="PSUM"))
        psT = phx.enter_context(tc.tile_pool(name="psT", bufs=2, space="PSUM"))

        # loads (x and w_adj cast to bf16 in-flight)
        x_sb = xp.tile([128, CT, HW], BF16)
        for ct in range(CT):
            nc.gpsimd.dma_start(out=x_sb[:, ct, :], in_=x_d[ct * 128:(ct + 1) * 128, :])
        w_def_sb = xp.tile([128, 2, C * K], BF16)      # cast-DMA fp32->bf16
        for ot in range(2):
            nc.gpsimd.dma_start(out=w_def_sb[:, ot, :],
                                in_=w_def_d[ot * 128:(ot + 1) * 128, :])
        w_adjT = offp.tile([128, CT, OFFC], BF16)
        for ct in range(CT):
            nc.gpsimd.dma_start(
                out=w_adjT[:, ct, :],
                in_=w_adj_d.rearrange("o c -> c o")[ct * 128:(ct + 1) * 128, :])
        b_adj_sb = offp.tile([OFFC, 1], FP32)
        nc.sync.dma_start(out=b_adj_sb[:], in_=b_adj_d[:, :])
        w_off_sb = offp.tile([OFFC, K], FP32)
        nc.sync.dma_start(out=w_off_sb[:], in_=w_off_d[:, :])
        b_off_sb = offp.tile([OFFC, 1], FP32)
        nc.sync.dma_start(out=b_off_sb[:], in_=b_off_d[:, :])

        # ---- phase 3: w_def transpose ----
        for kt in range(KT):
            k = kt // 2
            chalf = kt % 2
            for ot in range(2):
                ps = psW.tile([128, 128], BF16, tag="psw")
                src = w_def_sb[:, ot, :].rearrange("p (c k) -> p k c", k=K) \
                    [:, k, chalf * 128:(chalf + 1) * 128]
                nc.tensor.transpose(ps[:], src, ident_b[:])
                nc.scalar.copy(w_defT[:, kt, ot * 128:ot * 128 + 128], ps[:])

        # ---- phase 1: offsets pipeline ----
        GC = H + 2   # 66
        xch_pad = offp.tile([OFFC, GC * GC], BF16)
        nc.scalar.memzero(xch_pad[:])
        xch_v = xch_pad[:].rearrange("p (h w) -> p h w", h=GC, w=GC)
        for pch in range(8):
            ps = psA.tile([OFFC, 512], FP32)
            for ct in range(CT):
                nc.tensor.matmul(out=ps[:], lhsT=w_adjT[:, ct, :],
                                 rhs=x_sb[:, ct, pch * 512:(pch + 1) * 512],
                                 start=(ct == 0), stop=(ct == CT - 1))
            nc.scalar.activation(
                out=xch_v[:, 1 + pch * 8:1 + pch * 8 + 8, 1:1 + W],
                in_=ps[:].rearrange("p (h w) -> p h w", h=8, w=W),
                func=AF.Identity, bias=b_adj_sb[:], scale=1.0)

        # depthwise 3x3: taps 0-5 chained on DVE; taps 6-8 via Act scale
        # (Pool cannot run TensorScalarPtr), pairwise-summed on Pool.
        off_sb = offp.tile([OFFC, HW], BF16)
        ta = offp.tile([OFFC, 3, HW], BF16)
        ova = off_sb[:].rearrange("p (h w) -> p h w", h=H, w=W)
        for tap in range(6):
            di, dj = tap // 3, tap % 3
            vin = xch_v[:, di:di + H, dj:dj + W]
            if tap == 0:
                nc.vector.tensor_scalar(
                    out=ova, in0=vin, scalar1=w_off_sb[:, 0:1],
                    scalar2=b_off_sb[:, 0:1], op0=ALU.mult, op1=ALU.add)
            else:
                nc.vector.scalar_tensor_tensor(
                    out=ova, in0=vin, scalar=w_off_sb[:, tap:tap + 1],
                    in1=ova, op0=ALU.mult, op1=ALU.add)
        for tap in range(6, K):
            di, dj = tap // 3, tap % 3
            vin = xch_v[:, di:di + H, dj:dj + W]
            nc.scalar.activation(
                out=ta[:, tap - 6, :].rearrange("p (h w) -> p h w", h=H, w=W),
                in_=vin, func=AF.Identity, scale=w_off_sb[:, tap:tap + 1])
        nc.gpsimd.tensor_tensor(out=ta[:, 0, :], in0=ta[:, 0, :],
                                in1=ta[:, 1, :], op=ALU.add)
        nc.gpsimd.tensor_tensor(out=ta[:, 0, :], in0=ta[:, 0, :],
                                in1=ta[:, 2, :], op=ALU.add)
        nc.vector.tensor_tensor(out=off_sb[:], in0=off_sb[:], in1=ta[:, 0, :],
                                op=ALU.add)

        offT = scr.tile([128, NPT, OFFC], FP32)
        for t in range(NPT):
            pso = psT.tile([128, OFFC], BF16, tag="pst")
            nc.tensor.transpose(pso[:], off_sb[:, t * 128:(t + 1) * 128],
                                ident_b[:OFFC, :OFFC])
            nc.scalar.copy(offT[:, t, :], pso[:])

        dyv = offT[:].rearrange("p t (k two) -> p t k two", two=2)[:, :, :, 0]
        dxv = offT[:].rearrange("p t (k two) -> p t k two", two=2)[:, :, :, 1]
        py = scr.tile([128, NPT, K], FP32)
        px = scr.tile([128, NPT, K], FP32)
        nc.vector.scalar_tensor_tensor(out=py[:], in0=dyv, scalar=hh[:, 0:1],
                                       in1=by_f[:], op0=ALU.add, op1=ALU.add)
        nc.vector.scalar_tensor_tensor(out=px[:], in0=dxv, scalar=ww[:, 0:1],
                                       in1=bx_f[:], op0=ALU.add, op1=ALU.add)
        fyi = scr.tile([128, NPT, K], I32)
        fxi = scr.tile([128, NPT, K], I32)
        nc.vector.tensor_copy(fyi[:], py[:])
        nc.vector.tensor_copy(fxi[:], px[:])
        fy = scr.tile([128, NPT, K], FP32)
        fx = scr.tile([128, NPT, K], FP32)
        nc.vector.tensor_copy(fy[:], fyi[:])
        nc.vector.tensor_copy(fx[:], fxi[:])
        m = scr.tile([128, NPT, K], FP32)
        nc.vector.tensor_tensor(out=m[:], in0=fy[:], in1=py[:], op=ALU.is_gt)
        nc.vector.tensor_sub(out=fy[:], in0=fy[:], in1=m[:])
        nc.vector.tensor_tensor(out=m[:], in0=fx[:], in1=px[:], op=ALU.is_gt)
        nc.vector.tensor_sub(out=fx[:], in0=fx[:], in1=m[:])
        wv = wts_sb[:].rearrange("p t (k s) -> p t k s", s=3)
        nc.vector.tensor_sub(out=wv[:, :, :, 0], in0=px[:], in1=fx[:])
        nc.vector.tensor_sub(out=wv[:, :, :, 1], in0=py[:], in1=fy[:])
        nc.vector.tensor_tensor(out=wv[:, :, :, 2], in0=wv[:, :, :, 0],
                                in1=wv[:, :, :, 1], op=ALU.mult)
        r0f = scr.tile([128, NPT, K], FP32)
        nc.vector.scalar_tensor_tensor(out=r0f[:], in0=fy[:], scalar=float(G),
                                       in1=fx[:], op0=ALU.mult, op1=ALU.add)
        nc.vector.tensor_scalar(out=r0f[:], in0=r0f[:], scalar1=0.0,
                                scalar2=float(RT - G - 2), op0=ALU.max,
                                op1=ALU.min)
        nc.vector.tensor_copy(r0_sb[:, 0:16, :], r0f[:, 0:16, :])
        nc.vector.tensor_scalar(out=r0f[:, 16:, :], in0=r0f[:, 16:, :],
                                scalar1=-float(HB), scalar2=None, op0=ALU.add)
        nc.vector.tensor_copy(r0_sb[:, 16:, :], r0f[:, 16:, :])
        if GATHER == "multi":
            nc.vector.memset(r0T_f[:], 0.0)
            for j in range(3):
                nt = 12 if j < 2 else 8
                psr = psT.tile([108, 1], FP32, tag="psr")
                nc.tensor.transpose(psr[:9 * nt],
                                    r0f[0:1, 12 * j:12 * j + nt, :],
                                    ident_f[0:1, 0:1])
                nc.scalar.copy(r0T_f[:9 * nt, j:j + 1], psr[:9 * nt])
            r0T2_v = r0T2_f[:].rearrange("p (j w) -> p j w", w=12)
            for w in range(12):
                nj = 3 if w < 8 else 2
                nc.sync.dma_start(out=r0T2_v[:, 0:nj, w],
                                  in_=r0T_f[9 * w:9 * w + 9, 0:nj])

        # ---- phase 2: table build; Dy/Dxy per chunk ----
        xbf = tblp.tile([128, CT, RT], BF16)
        dxf = tblp.tile([128, CT, RT], BF16)
        head = PAD * G + PAD
        for ct in range(CT):
            # zero: head band, inter-row 8-col gaps, tail band
            nc.vector.memset(xbf[:, ct, 0:head], 0.0)
            nc.vector.memset(
                xbf[:, ct, head + W:head + W + (H - 1) * G]
                    .rearrange("p (r q) -> p r q", q=G)[:, :, 0:G - W], 0.0)
            nc.vector.memset(xbf[:, ct, head + (H - 1) * G + W:], 0.0)
            nc.vector.tensor_copy(
                xbf[:, ct, :ROWS].rearrange("p (h w) -> p h w", h=G, w=G)
                    [:, PAD:PAD + H, PAD:PAD + W],
                x_sb[:, ct, :].rearrange("p (h w) -> p h w", h=H, w=W))
        for ct in range(CT):
            nc.vector.tensor_sub(out=dxf[:, ct, 0:RT - 1],
                                 in0=xbf[:, ct, 1:RT], in1=xbf[:, ct, 0:RT - 1])
            nc.gpsimd.memset(dxf[:, ct, RT - 1:RT], 0.0)

        for band in range(2):
          btab = table_lo if band == 0 else table_hi
          for rt in range(22):
            lo = (0 if band == 0 else HB) + rt * 128
            n_y = min(128, RT - G - lo)    # rows with r+G in range
            tb = evb.tile([128, 4, C], BF16, tag="tb")
            for ct in range(CT):
                dyc = dyb.tile([128, 2, 128], BF16, tag="dy")
                e1 = nc.vector if band == 1 else nc.gpsimd
                if n_y > 0:
                    nc.vector.tensor_tensor(out=dyc[:, 0, :n_y],
                                            in0=xbf[:, ct, lo + G:lo + G + n_y],
                                            in1=xbf[:, ct, lo:lo + n_y],
                                            op=ALU.subtract)
                    e1.tensor_tensor(out=dyc[:, 1, :n_y],
                                     in0=dxf[:, ct, lo + G:lo + G + n_y],
                                     in1=dxf[:, ct, lo:lo + n_y],
                                     op=ALU.subtract)
                if n_y < 128:
                    nc.vector.memset(dyc[:, 0, max(n_y, 0):], 0.0)
                    nc.vector.memset(dyc[:, 1, max(n_y, 0):], 0.0)
                ps = psB.tile([128, 4 * 128], BF16, tag="ps")
                nc.tensor.transpose(ps[:, 0:128], xbf[:, ct, lo:lo + 128], ident_b[:])
                nc.tensor.transpose(ps[:, 128:256], dxf[:, ct, lo:lo + 128], ident_b[:])
                nc.tensor.transpose(ps[:, 256:384], dyc[:, 0, :], ident_b[:])
                nc.tensor.transpose(ps[:, 384:512], dyc[:, 1, :], ident_b[:])
                tbv = tb[:, :, ct * 128:(ct + 1) * 128]
                psv = ps[:].rearrange("p (s c) -> p s c", s=4)
                if (rt + ct) % 2 == 0:
                    nc.scalar.copy(tbv, psv)
                else:
                    nc.vector.tensor_copy(tbv, psv)
            nc.sync.dma_start(out=btab[rt * 128:(rt + 1) * 128, :], in_=tb[:])
        phx.close()

        # ---------------- main loop ----------------
        outp = ctx.enter_context(tc.tile_pool(name="outp", bufs=1))
        out_sb = outp.tile([128, 2, HW], FP32)
        with tc.tile_pool(name="gat", bufs=int(os.environ.get("GBUFS", "3"))) as gat, \
             tc.tile_pool(name="dia", bufs=2) as dia, \
             tc.tile_pool(name="fixp", bufs=2) as fixp, \
             tc.tile_pool(name="smp", bufs=2) as smp, \
             tc.tile_pool(name="psS", bufs=int(os.environ.get("PSBUFS", "3")), space="PSUM") as psS, \
             tc.tile_pool(name="psO", bufs=2, space="PSUM") as psO:
            for t in range(NPT):
                g_sb = gat.tile([128, K, 4 * C], BF16, tag="g")
                tabt = table_lo if t < 16 else table_hi
                if GATHER == "multi":
                    nc.gpsimd.indirect_dma_start(
                        out=g_sb[:], out_offset=None,
                        in_=tabt[:, :],
                        in_offset=IndirectOffsetOnAxis(ap=r0_sb[:, t, :], axis=0),
                        bounds_check=BND - 1, oob_is_err=False)
                    idxf_f = fixp.tile([K, 1], FP32, tag="ixf")
                    nc.vector.scalar_tensor_tensor(
                        out=idxf_f[:], in0=g_sb[0:K, 0, 0:1], scalar=0.0,
                        in1=r0T2_f[:, t:t + 1],
                        op0=ALU.mult, op1=ALU.add)
                    idxf_i = fixp.tile([K, 1], I32, tag="ixi")
                    nc.vector.tensor_copy(idxf_i[:], idxf_f[:])
                    gF = fixp.tile([K, 4 * C], BF16, tag="gf")
                    nc.gpsimd.indirect_dma_start(
                        out=gF[:], out_offset=None,
                        in_=tabt[:, :],
                        in_offset=IndirectOffsetOnAxis(ap=idxf_i[:, :], axis=0))
                    nc.sync.dma_start(out=g_sb[0:1, :, :], in_=gF[:, :])
                else:
                    for k in range(K):
                        nc.gpsimd.indirect_dma_start(
                            out=g_sb[:, k, :], out_offset=None,
                            in_=tabt[:, :],
                            in_offset=IndirectOffsetOnAxis(ap=r0_sb[:, t, k:k + 1], axis=0))

                dg = dia.tile([128, K * 3, 128], BF16, tag="d")
                for k in range(K):
                    for s in range(3):
                        nc.vector.tensor_scalar(
                            out=dg[:, 3 * k + s, :], in0=ident_b[:],
                            scalar1=wts_sb[:, t, 3 * k + s:3 * k + s + 1],
                            scalar2=None, op0=ALU.mult)

                sampT = smp.tile([128, KT, 128], BF16, tag="st")
                for q in range(5):
                    n_in_g = 4 if q < 4 else 2
                    ps = psS.tile([128, 4 * 128], FP32, tag="pss")
                    for j in range(n_in_g):
                        kt = q * 4 + j
                        k = kt // 2
                        ch = kt % 2
                        win = ps[:, j * 128:(j + 1) * 128]
                        gv = g_sb[:, k, :].rearrange("p (s c) -> p s c", s=4)
                        nc.tensor.matmul(
                            out=win, lhsT=gv[:, 0, ch * 128:(ch + 1) * 128],
                            rhs=ident_b[:], start=True, stop=False)
                        for s in range(3):
                            nc.tensor.matmul(
                                out=win, lhsT=gv[:, s + 1, ch * 128:(ch + 1) * 128],
                                rhs=dg[:, 3 * k + s, :], start=False, stop=(s == 2))
                    nc.scalar.copy(sampT[:, q * 4:q * 4 + n_in_g, :],
                                   ps[:, :n_in_g * 128])

                for ot in range(2):
                    pso = psO.tile([128, 128], FP32, tag="po")
                    for kt in range(KT):
                        nc.tensor.matmul(out=pso[:],
                                         lhsT=w_defT[:, kt, ot * 128:(ot + 1) * 128],
                                         rhs=sampT[:, kt, :],
                                         start=(kt == 0), stop=(kt == KT - 1))
                    nc.scalar.copy(out_sb[:, ot, t * 128:(t + 1) * 128], pso[:])
                if t % 8 == 7:
                    for ot in range(2):
                        nc.sync.dma_start(
                            out=out_d[ot * 128:(ot + 1) * 128,
                                      (t - 7) * 128:(t + 1) * 128],
                            in_=out_sb[:, ot, (t - 7) * 128:(t + 1) * 128])
    return nc


_CACHE = {}


def _get_nc():
    if "nc" not in _CACHE:
        nc = build_nc()
        if not nc.is_finalized():
            nc.finalize()
        _CACHE["nc"] = nc
    return _CACHE["nc"]


def kernel(**inputs):
    from concourse import bass_utils
    x = np.ascontiguousarray(inputs["x"], dtype=np.float32)          # [8,256,64,64]
    w_adj = np.ascontiguousarray(inputs["w_adj"], dtype=np.float32).reshape(OFFC, C)
    b_adj = np.ascontiguousarray(inputs["b_adj"], dtype=np.float32).reshape(OFFC, 1)
    w_off = np.ascontiguousarray(inputs["w_off"], dtype=np.float32).reshape(OFFC, K)
    b_off = np.ascontiguousarray(inputs["b_off"], dtype=np.float32).reshape(OFFC, 1)
    w_def = np.ascontiguousarray(inputs["w_def"], dtype=np.float32).reshape(C, C * K)

    nc = _get_nc()
    in_maps = []
    for n in range(N):
        in_maps.append({
            "x": np.ascontiguousarray(x[n].reshape(C, HW)),
            "w_adj": w_adj, "b_adj": b_adj,
            "w_off": w_off, "b_off": b_off,
            "w_def": w_def,
        })
    res = bass_utils.run_bass_kernel_spmd(nc, in_maps, core_ids=list(range(N)))
    outs = [res.results[n]["out"].reshape(C, H, W) for n in range(N)]
    return np.stack(outs, axis=0)


if __name__ == "__main__":
    nc = build_nc()
    print("build ok")


# revision 4
# speedup vs baseline: 1.5956x; 1.0728x over previous
"""Deformable conv (nn_DeformConv) Trainium2 Bass kernel, v3.

Per core = one batch element, data-parallel across 8 cores.

Phases 1 (offsets), 2 (table build), 3 (w_def transpose) share one pool
scope so the tile scheduler can overlap them freely; all close before the
main loop.

Main loop per 128-position tile:
  - 9 per-tap indirect row gathers from the DRAM table
    [x | Dx | Dy | Dxy] (2KB bf16 rows)  [GATHER=tap]
    or 1 multi-idx gather + token + fixup-gather + fixup DMA [GATHER=multi].
  - combine+transpose fused on PE: per (tap, chalf) accumulate in PSUM
    T(x) + T(Dx)·diag(rx) + T(Dy)·diag(ry) + T(Dxy)·diag(rxry);
    diag mats built per tile by DVE tensor_scalar (4x perf mode).
  - PSUM-accumulated matmul against w_defT -> out, out DMA per 8 tiles.
"""
import os
import numpy as np
from contextlib import ExitStack

import concourse.bass as bass
import concourse.mybir as mybir
import concourse.tile as tile
from concourse import bacc as _bacc
from concourse.bass import IndirectOffsetOnAxis
from concourse.masks import make_identity

FP32 = mybir.dt.float32
BF16 = mybir.dt.bfloat16
I32 = mybir.dt.int32

N, C, H, W = 8, 256, 64, 64
HW = H * W                    # 4096
K = 9
OFFC = 18
PAD = 4
G = H + 2 * PAD               # 72
ROWS = G * G                  # 5184
RT = 5248                     # rows padded to 41*128
NRT = RT // 128               # 41
NPT = HW // 128               # 32 position tiles
CT = C // 128                 # 2 channel tiles
KT = (C * K) // 128           # 18 contraction tiles
ALU = mybir.AluOpType
AF = mybir.ActivationFunctionType

GATHER = os.environ.get("GATHER", "tap")   # tap | multi


def build_nc():
    nc = _bacc.Bacc()
    x_d = nc.dram_tensor("x", [C, HW], FP32, kind="ExternalInput")
    w_adj_d = nc.dram_tensor("w_adj", [OFFC, C], FP32, kind="ExternalInput")
    b_adj_d = nc.dram_tensor("b_adj", [OFFC, 1], FP32, kind="ExternalInput")
    w_off_d = nc.dram_tensor("w_off", [OFFC, K], FP32, kind="ExternalInput")
    b_off_d = nc.dram_tensor("b_off", [OFFC, 1], FP32, kind="ExternalInput")
    w_def_d = nc.dram_tensor("w_def", [C, C * K], FP32, kind="ExternalInput")
    out_d = nc.dram_tensor("out", [C, HW], FP32, kind="ExternalOutput")

    with tile.TileContext(nc) as tc, ExitStack() as ctx:
        pers = ctx.enter_context(tc.tile_pool(name="pers", bufs=1))
        dram = ctx.enter_context(tc.tile_pool(name="dram", bufs=1, space="DRAM"))

        # 4 row-bands: gathers for tiles 8g..8g+7 need only band g, so the
        # main loop starts after ~13 chunks instead of the full table.
        BANDS = [(0, 13), (1280, 12), (2432, 12), (3584, 13)]
        tables = []
        for _bi, (_, _nch) in enumerate(BANDS):
            tables.append(dram.tile([_nch * 128, 4 * C], BF16, name=f"tband{_bi}"))

        ident_f = pers.tile([128, 128], FP32)
        make_identity(nc, ident_f[:])
        ident_b = pers.tile([128, 128], BF16)
        nc.vector.tensor_copy(ident_b[:], ident_f[:])

        # per-partition constants: hh = p//64 (0/1), ww = p%64
        iota_p = pers.tile([128, 1], I32)
        nc.gpsimd.iota(iota_p[:], pattern=[[0, 1]], base=0, channel_multiplier=1)
        pf = pers.tile([128, 1], FP32)
        nc.vector.tensor_copy(pf[:], iota_p[:])
        hh = pers.tile([128, 1], FP32)
        nc.vector.tensor_scalar(out=hh[:], in0=pf[:], scalar1=64.0, scalar2=None,
                                op0=ALU.is_ge)
        ww = pers.tile([128, 1], FP32)
        nc.vector.scalar_tensor_tensor(out=ww[:], in0=hh[:], scalar=-64.0,
                                       in1=pf[:], op0=ALU.mult, op1=ALU.add)

        # batched base ramps over (t, k): by = 2t + ki + (PAD-1), bx = kj + (PAD-1)
        by_i = pers.tile([128, NPT, K], I32)
        nc.gpsimd.iota(by_i[:], pattern=[[2, NPT], [1, 3], [0, 3]], base=PAD - 1,
                       channel_multiplier=0)
        bx_i = pers.tile([128, NPT, K], I32)
        nc.gpsimd.iota(bx_i[:], pattern=[[0, NPT], [0, 3], [1, 3]], base=PAD - 1,
                       channel_multiplier=0)
        by_f = pers.tile([128, NPT, K], FP32)
        nc.vector.tensor_copy(by_f[:], by_i[:])
        bx_f = pers.tile([128, NPT, K], FP32)
        nc.vector.tensor_copy(bx_f[:], bx_i[:])

        w_defT = pers.tile([128, KT, 2 * 128], BF16)   # [ck-part, kt, o]
        r0_sb = pers.tile([128, NPT, K], I32)
        wts_sb = pers.tile([128, NPT, K * 3], FP32)    # k-major (rx, ry, rxry)
        r0T_f = pers.tile([108, 3], FP32)              # fixup staging
        r0T2_f = pers.tile([9, 36], FP32)              # r0T2_f[k, t]

        # ---------------- phases 1+3 scope (freed before main loop);
        # phase-2 pools stay open so the hi-band build overlaps the loop ----
        tblp = ctx.enter_context(tc.tile_pool(name="tblp", bufs=1))
        evb = ctx.enter_context(tc.tile_pool(name="evb", bufs=3))
        dyb = ctx.enter_context(tc.tile_pool(name="dyb", bufs=3))
        psB = ctx.enter_context(tc.tile_pool(name="psB", bufs=3, space="PSUM"))
        phx = ExitStack()
        xp = phx.enter_context(tc.tile_pool(name="xp", bufs=1))
        offp = phx.enter_context(tc.tile_pool(name="offp", bufs=1))
        scr = phx.enter_context(tc.tile_pool(name="scr", bufs=1))
        psW = phx.enter_context(tc.tile_pool(name="psW", bufs=1, space="PSUM"))
        psA = phx.enter_context(tc.tile_pool(name="psA", bufs=2, space="PSUM"))
        psT = phx.enter_context(tc.tile_pool(name="psT", bufs=2, space="PSUM"))

        # loads (x and w_adj cast to bf16 in-flight)
        x_sb = xp.tile([128, CT, HW], BF16)
        for ct in range(CT):
            nc.gpsimd.dma_start(out=x_sb[:, ct, :], in_=x_d[ct * 128:(ct + 1) * 128, :])
        w_adjT = offp.tile([128, CT, OFFC], BF16)
        for ct in range(CT):
            nc.gpsimd.dma_start(
                out=w_adjT[:, ct, :],
                in_=w_adj_d.rearrange("o c -> c o")[ct * 128:(ct + 1) * 128, :])
        b_adj_sb = offp.tile([OFFC, 1], FP32)
        nc.sync.dma_start(out=b_adj_sb[:], in_=b_adj_d[:, :])
        w_off_sb = offp.tile([OFFC, K], FP32)
        nc.sync.dma_start(out=w_off_sb[:], in_=w_off_d[:, :])
        b_off_sb = offp.tile([OFFC, 1], FP32)
        nc.sync.dma_start(out=b_off_sb[:], in_=b_off_d[:, :])

        # ---- phase 1: offsets pipeline ----
        GC = H + 2   # 66
        xch_pad = offp.tile([OFFC, GC * GC], BF16)
        nc.scalar.memzero(xch_pad[:])
        xch_v = xch_pad[:].rearrange("p (h w) -> p h w", h=GC, w=GC)
        for pch in range(8):
            ps = psA.tile([OFFC, 512], FP32)
            for ct in range(CT):
                nc.tensor.matmul(out=ps[:], lhsT=w_adjT[:, ct, :],
                                 rhs=x_sb[:, ct, pch * 512:(pch + 1) * 512],
                                 start=(ct == 0), stop=(ct == CT - 1))
            nc.scalar.activation(
                out=xch_v[:, 1 + pch * 8:1 + pch * 8 + 8, 1:1 + W],
                in_=ps[:].rearrange("p (h w) -> p h w", h=8, w=W),
                func=AF.Identity, bias=b_adj_sb[:], scale=1.0)

        # depthwise 3x3 on a 108-partition layout: partition = (row-block, ch),
        # 6 row-blocks of <=11 output rows, so each DVE op is [108, ~704]
        # instead of [18, 4096] (7x less time on the serial 9-tap chain).
        BLK = [(0, 11), (11, 11), (22, 11), (33, 11), (44, 11), (55, 9)]
        # scalar tiles replicated per block: w_off/b_off at partitions 18b+ch
        w_off6 = offp.tile([108, K], FP32)
        b_off6 = offp.tile([108, 1], FP32)
        for b in range(6):
            nc.sync.dma_start(out=w_off6[18 * b:18 * b + 18, :], in_=w_off_d[:, :])
            nc.sync.dma_start(out=b_off6[18 * b:18 * b + 18, :], in_=b_off_d[:, :])
        x2 = offp.tile([108, 13 * GC], BF16)
        nc.vector.memset(x2[:], 0.0)
        for b, (r0b, nr) in enumerate(BLK):
            nc.sync.dma_start(out=x2[18 * b:18 * b + 18, 0:(nr + 2) * GC],
                              in_=xch_pad[:, r0b * GC:(r0b + nr + 2) * GC])
        o2 = offp.tile([108, 11 * W], BF16)
        x2v = x2[:].rearrange("p (h w) -> p h w", h=13, w=GC)
        o2v = o2[:].rearrange("p (h w) -> p h w", h=11, w=W)
        nrmax = 11
        for tap in range(K):
            di, dj = tap // 3, tap % 3
            vin = x2v[:, di:di + nrmax, dj:dj + W]
            if tap == 0:
                nc.vector.tensor_scalar(
                    out=o2v, in0=vin, scalar1=w_off6[:, 0:1],
                    scalar2=b_off6[:, 0:1], op0=ALU.mult, op1=ALU.add)
            else:
                nc.vector.scalar_tensor_tensor(
                    out=o2v, in0=vin, scalar=w_off6[:, tap:tap + 1],
                    in1=o2v, op0=ALU.mult, op1=ALU.add)
        off_sb = offp.tile([OFFC, HW], BF16)
        for b, (r0b, nr) in enumerate(BLK):
            nc.sync.dma_start(out=off_sb[:, r0b * W:(r0b + nr) * W],
                              in_=o2[18 * b:18 * b + 18, 0:nr * W])

        offT = scr.tile([128, NPT, OFFC], FP32)
        for t in range(NPT):
            pso = psT.tile([128, OFFC], BF16, tag="pst")
            nc.tensor.transpose(pso[:], off_sb[:, t * 128:(t + 1) * 128],
                                ident_b[:OFFC, :OFFC])
            nc.scalar.copy(offT[:, t, :], pso[:])

        dyv = offT[:].rearrange("p t (k two) -> p t k two", two=2)[:, :, :, 0]
        dxv = offT[:].rearrange("p t (k two) -> p t k two", two=2)[:, :, :, 1]
        py = scr.tile([128, NPT, K], FP32)
        px = scr.tile([128, NPT, K], FP32)
        nc.vector.scalar_tensor_tensor(out=py[:], in0=dyv, scalar=hh[:, 0:1],
                                       in1=by_f[:], op0=ALU.add, op1=ALU.add)
        nc.vector.scalar_tensor_tensor(out=px[:], in0=dxv, scalar=ww[:, 0:1],
                                       in1=bx_f[:], op0=ALU.add, op1=ALU.add)
        fyi = scr.tile([128, NPT, K], I32)
        fxi = scr.tile([128, NPT, K], I32)
        nc.vector.tensor_copy(fyi[:], py[:])
        nc.vector.tensor_copy(fxi[:], px[:])
        fy = scr.tile([128, NPT, K], FP32)
        fx = scr.tile([128, NPT, K], FP32)
        nc.vector.tensor_copy(fy[:], fyi[:])
        nc.vector.tensor_copy(fx[:], fxi[:])
        m = scr.tile([128, NPT, K], FP32)
        nc.vector.tensor_tensor(out=m[:], in0=fy[:], in1=py[:], op=ALU.is_gt)
        nc.vector.tensor_sub(out=fy[:], in0=fy[:], in1=m[:])
        nc.vector.tensor_tensor(out=m[:], in0=fx[:], in1=px[:], op=ALU.is_gt)
        nc.vector.tensor_sub(out=fx[:], in0=fx[:], in1=m[:])
        wv = wts_sb[:].rearrange("p t (k s) -> p t k s", s=3)
        nc.vector.tensor_sub(out=wv[:, :, :, 0], in0=px[:], in1=fx[:])
        nc.vector.tensor_sub(out=wv[:, :, :, 1], in0=py[:], in1=fy[:])
        nc.vector.tensor_tensor(out=wv[:, :, :, 2], in0=wv[:, :, :, 0],
                                in1=wv[:, :, :, 1], op=ALU.mult)
        r0f = scr.tile([128, NPT, K], FP32)
        nc.vector.scalar_tensor_tensor(out=r0f[:], in0=fy[:], scalar=float(G),
                                       in1=fx[:], op0=ALU.mult, op1=ALU.add)
        nc.vector.tensor_scalar(out=r0f[:], in0=r0f[:], scalar1=0.0,
                                scalar2=float(RT - G - 2), op0=ALU.max,
                                op1=ALU.min)
        nc.vector.tensor_copy(r0_sb[:, 0:8, :], r0f[:, 0:8, :])
        for gb in range(1, 4):
            nc.vector.tensor_scalar(
                out=r0f[:, 8 * gb:8 * gb + 8, :], in0=r0f[:, 8 * gb:8 * gb + 8, :],
                scalar1=-float(BANDS[gb][0]), scalar2=None, op0=ALU.add)
            nc.vector.tensor_copy(r0_sb[:, 8 * gb:8 * gb + 8, :],
                                  r0f[:, 8 * gb:8 * gb + 8, :])
        if GATHER == "multi":
            nc.vector.memset(r0T_f[:], 0.0)
            for j in range(3):
                nt = 12 if j < 2 else 8
                psr = psT.tile([108, 1], FP32, tag="psr")
                nc.tensor.transpose(psr[:9 * nt],
                                    r0f[0:1, 12 * j:12 * j + nt, :],
                                    ident_f[0:1, 0:1])
                nc.scalar.copy(r0T_f[:9 * nt, j:j + 1], psr[:9 * nt])
            r0T2_v = r0T2_f[:].rearrange("p (j w) -> p j w", w=12)
            for w in range(12):
                nj = 3 if w < 8 else 2
                nc.sync.dma_start(out=r0T2_v[:, 0:nj, w],
                                  in_=r0T_f[9 * w:9 * w + 9, 0:nj])

        w_def_sb = xp.tile([128, 2, C * K], BF16)      # cast-DMA fp32->bf16
        for ot in range(2):
            nc.gpsimd.dma_start(out=w_def_sb[:, ot, :],
                                in_=w_def_d[ot * 128:(ot + 1) * 128, :])
        # ---- phase 3: w_def transpose ----
        for kt in range(KT):
            k = kt // 2
            chalf = kt % 2
            for ot in range(2):
                ps = psW.tile([128, 128], BF16, tag="psw")
                src = w_def_sb[:, ot, :].rearrange("p (c k) -> p k c", k=K) \
                    [:, k, chalf * 128:(chalf + 1) * 128]
                nc.tensor.transpose(ps[:], src, ident_b[:])
                nc.scalar.copy(w_defT[:, kt, ot * 128:ot * 128 + 128], ps[:])

        # ---- phase 2: table build; Dy/Dxy per chunk ----
        xbf = tblp.tile([128, CT, RT], BF16)
        dxf = tblp.tile([128, CT, RT], BF16)
        head = PAD * G + PAD
        for ct in range(CT):
            # zero: head band, inter-row 8-col gaps, tail band
            nc.vector.memset(xbf[:, ct, 0:head], 0.0)
            nc.vector.memset(
                xbf[:, ct, head + W:head + W + (H - 1) * G]
                    .rearrange("p (r q) -> p r q", q=G)[:, :, 0:G - W], 0.0)
            nc.vector.memset(xbf[:, ct, head + (H - 1) * G + W:], 0.0)
            nc.vector.tensor_copy(
                xbf[:, ct, :ROWS].rearrange("p (h w) -> p h w", h=G, w=G)
                    [:, PAD:PAD + H, PAD:PAD + W],
                x_sb[:, ct, :].rearrange("p (h w) -> p h w", h=H, w=W))
        for ct in range(CT):
            nc.vector.tensor_sub(out=dxf[:, ct, 0:RT - 1],
                                 in0=xbf[:, ct, 1:RT], in1=xbf[:, ct, 0:RT - 1])
            nc.gpsimd.memset(dxf[:, ct, RT - 1:RT], 0.0)

        for band in range(4):
          btab = tables[band]
          for rt in range(BANDS[band][1]):
            lo = BANDS[band][0] + rt * 128
            n_y = min(128, RT - G - lo)    # rows with r+G in range
            tb = evb.tile([128, 4, C], BF16, tag="tb")
            for ct in range(CT):
                dyc = dyb.tile([128, 2, 128], BF16, tag="dy")
                e1 = nc.vector if band >= 1 else nc.gpsimd
                if n_y > 0:
                    e1.tensor_tensor(out=dyc[:, 0, :n_y],
                                     in0=xbf[:, ct, lo + G:lo + G + n_y],
                                     in1=xbf[:, ct, lo:lo + n_y],
                                     op=ALU.subtract)
                    e1.tensor_tensor(out=dyc[:, 1, :n_y],
                                     in0=dxf[:, ct, lo + G:lo + G + n_y],
                                     in1=dxf[:, ct, lo:lo + n_y],
                                     op=ALU.subtract)
                if n_y < 128:
                    nc.vector.memset(dyc[:, 0, max(n_y, 0):], 0.0)
                    nc.vector.memset(dyc[:, 1, max(n_y, 0):], 0.0)
                ps = psB.tile([128, 4 * 128], BF16, tag="ps")
                nc.tensor.transpose(ps[:, 0:128], xbf[:, ct, lo:lo + 128], ident_b[:])
                nc.tensor.transpose(ps[:, 128:256], dxf[:, ct, lo:lo + 128], ident_b[:])
                nc.tensor.transpose(ps[:, 256:384], dyc[:, 0, :], ident_b[:])
                nc.tensor.transpose(ps[:, 384:512], dyc[:, 1, :], ident_b[:])
                tbv = tb[:, :, ct * 128:(ct + 1) * 128]
                psv = ps[:].rearrange("p (s c) -> p s c", s=4)
                if (rt + ct) % 2 == 0:
                    nc.scalar.copy(tbv, psv)
                else:
                    nc.vector.tensor_copy(tbv, psv)
            nc.sync.dma_start(out=btab[rt * 128:(rt + 1) * 128, :], in_=tb[:])
        phx.close()

        # ---------------- main loop ----------------
        outp = ctx.enter_context(tc.tile_pool(name="outp", bufs=1))
        out_sb = outp.tile([128, 2, HW], FP32)
        with tc.tile_pool(name="gat", bufs=int(os.environ.get("GBUFS", "3"))) as gat, \
             tc.tile_pool(name="dia", bufs=2) as dia, \
             tc.tile_pool(name="fixp", bufs=2) as fixp, \
             tc.tile_pool(name="smp", bufs=2) as smp, \
             tc.tile_pool(name="psS", bufs=int(os.environ.get("PSBUFS", "3")), space="PSUM") as psS, \
             tc.tile_pool(name="psO", bufs=2, space="PSUM") as psO:
            for t in range(NPT):
                g_sb = gat.tile([128, K, 4 * C], BF16, tag="g")
                tabt = tables[t // 8]
                if GATHER == "multi":
                    nc.gpsimd.indirect_dma_start(
                        out=g_sb[:], out_offset=None,
                        in_=tabt[:, :],
                        in_offset=IndirectOffsetOnAxis(ap=r0_sb[:, t, :], axis=0),
                        bounds_check=BND - 1, oob_is_err=False)
                    idxf_f = fixp.tile([K, 1], FP32, tag="ixf")
                    nc.vector.scalar_tensor_tensor(
                        out=idxf_f[:], in0=g_sb[0:K, 0, 0:1], scalar=0.0,
                        in1=r0T2_f[:, t:t + 1],
                        op0=ALU.mult, op1=ALU.add)
                    idxf_i = fixp.tile([K, 1], I32, tag="ixi")
                    nc.vector.tensor_copy(idxf_i[:], idxf_f[:])
                    gF = fixp.tile([K, 4 * C], BF16, tag="gf")
                    nc.gpsimd.indirect_dma_start(
                        out=gF[:], out_offset=None,
                        in_=tabt[:, :],
                        in_offset=IndirectOffsetOnAxis(ap=idxf_i[:, :], axis=0))
                    nc.sync.dma_start(out=g_sb[0:1, :, :], in_=gF[:, :])
                else:
                    for k in range(K):
                        nc.gpsimd.indirect_dma_start(
                            out=g_sb[:, k, :], out_offset=None,
                            in_=tabt[:, :],
                            in_offset=IndirectOffsetOnAxis(ap=r0_sb[:, t, k:k + 1], axis=0))

                dg = dia.tile([128, K * 3, 128], BF16, tag="d")
                for k in range(K):
                    for s in range(3):
                        nc.vector.tensor_scalar(
                            out=dg[:, 3 * k + s, :], in0=ident_b[:],
                            scalar1=wts_sb[:, t, 3 * k + s:3 * k + s + 1],
                            scalar2=None, op0=ALU.mult)

                sampT = smp.tile([128, KT, 128], BF16, tag="st")
                for q in range(5):
                    n_in_g = 4 if q < 4 else 2
                    ps = psS.tile([128, 4 * 128], FP32, tag="pss")
                    for j in range(n_in_g):
                        kt = q * 4 + j
                        k = kt // 2
                        ch = kt % 2
                        win = ps[:, j * 128:(j + 1) * 128]
                        gv = g_sb[:, k, :].rearrange("p (s c) -> p s c", s=4)
                        nc.tensor.matmul(
                            out=win, lhsT=gv[:, 0, ch * 128:(ch + 1) * 128],
                            rhs=ident_b[:], start=True, stop=False)
                        for s in range(3):
                            nc.tensor.matmul(
                                out=win, lhsT=gv[:, s + 1, ch * 128:(ch + 1) * 128],
                                rhs=dg[:, 3 * k + s, :], start=False, stop=(s == 2))
                    nc.scalar.copy(sampT[:, q * 4:q * 4 + n_in_g, :],
                                   ps[:, :n_in_g * 128])

                for ot in range(2):
                    pso = psO.tile([128, 128], FP32, tag="po")
                    for kt in range(KT):
                        nc.tensor.matmul(out=pso[:],
                                         lhsT=w_defT[:, kt, ot * 128:(ot + 1) * 128],
                                         rhs=sampT[:, kt, :],
                                         start=(kt == 0), stop=(kt == KT - 1))
                    nc.scalar.copy(out_sb[:, ot, t * 128:(t + 1) * 128], pso[:])
                ogrp = [7, 15, 23, 27, 29, 30, 31]
                if t in ogrp:
                    t0o = ([-1] + ogrp)[ogrp.index(t)] + 1
                    for ot in range(2):
                        nc.sync.dma_start(
                            out=out_d[ot * 128:(ot + 1) * 128,
                                      t0o * 128:(t + 1) * 128],
                            in_=out_sb[:, ot, t0o * 128:(t + 1) * 128])
    return nc


_CACHE = {}


def _get_nc():
    if "nc" not in _CACHE:
        nc = build_nc()
        if not nc.is_finalized():
            nc.finalize()
        _CACHE["nc"] = nc
    return _CACHE["nc"]


def kernel(**inputs):
    from concourse import bass_utils
    x = np.ascontiguousarray(inputs["x"], dtype=np.float32)          # [8,256,64,64]
    w_adj = np.ascontiguousarray(inputs["w_adj"], dtype=np.float32).reshape(OFFC, C)
    b_adj = np.ascontiguousarray(inputs["b_adj"], dtype=np.float32).reshape(OFFC, 1)
    w_off = np.ascontiguousarray(inputs["w_off"], dtype=np.float32).reshape(OFFC, K)
    b_off = np.ascontiguousarray(inputs["b_off"], dtype=np.float32).reshape(OFFC, 1)
    w_def = np.ascontiguousarray(inputs["w_def"], dtype=np.float32).reshape(C, C * K)

    nc = _get_nc()
    in_maps = []
    for n in range(N):
        in_maps.append({
            "x": np.ascontiguousarray(x[n].reshape(C, HW)),
            "w_adj": w_adj, "b_adj": b_adj,
            "w_off": w_off, "b_off": b_off,
            "w_def": w_def,
        })
    res = bass_utils.run_bass_kernel_spmd(nc, in_maps, core_ids=list(range(N)))
    outs = [res.results[n]["out"].reshape(C, H, W) for n in range(N)]
    return np.stack(outs, axis=0)


if __name__ == "__main__":
    nc = build_nc()
    print("build ok")


# revision 5
# speedup vs baseline: 1.6305x; 1.0219x over previous
"""Deformable conv (nn_DeformConv) Trainium2 Bass kernel, v3.

Per core = one batch element, data-parallel across 8 cores.

Phases 1 (offsets), 2 (table build), 3 (w_def transpose) share one pool
scope so the tile scheduler can overlap them freely; all close before the
main loop.

Main loop per 128-position tile:
  - 9 per-tap indirect row gathers from the DRAM table
    [x | Dx | Dy | Dxy] (2KB bf16 rows)  [GATHER=tap]
    or 1 multi-idx gather + token + fixup-gather + fixup DMA [GATHER=multi].
  - combine+transpose fused on PE: per (tap, chalf) accumulate in PSUM
    T(x) + T(Dx)·diag(rx) + T(Dy)·diag(ry) + T(Dxy)·diag(rxry);
    diag mats built per tile by DVE tensor_scalar (4x perf mode).
  - PSUM-accumulated matmul against w_defT -> out, out DMA per 8 tiles.
"""
import os
import numpy as np
from contextlib import ExitStack

import concourse.bass as bass
import concourse.mybir as mybir
import concourse.tile as tile
from concourse import bacc as _bacc
from concourse.bass import IndirectOffsetOnAxis
from concourse.masks import make_identity

FP32 = mybir.dt.float32
BF16 = mybir.dt.bfloat16
I32 = mybir.dt.int32

N, C, H, W = 8, 256, 64, 64
HW = H * W                    # 4096
K = 9
OFFC = 18
PAD = 4
G = H + 2 * PAD               # 72
ROWS = G * G                  # 5184
RT = 5248                     # rows padded to 41*128
NRT = RT // 128               # 41
NPT = HW // 128               # 32 position tiles
CT = C // 128                 # 2 channel tiles
KT = (C * K) // 128           # 18 contraction tiles
ALU = mybir.AluOpType
AF = mybir.ActivationFunctionType

GATHER = os.environ.get("GATHER", "tap")   # tap | multi


def build_nc():
    nc = _bacc.Bacc()
    x_d = nc.dram_tensor("x", [C, HW], FP32, kind="ExternalInput")
    w_adj_d = nc.dram_tensor("w_adj", [OFFC, C], FP32, kind="ExternalInput")
    b_adj_d = nc.dram_tensor("b_adj", [OFFC, 1], FP32, kind="ExternalInput")
    w_off_d = nc.dram_tensor("w_off", [OFFC, K], FP32, kind="ExternalInput")
    b_off_d = nc.dram_tensor("b_off", [OFFC, 1], FP32, kind="ExternalInput")
    w_def_d = nc.dram_tensor("w_def", [C, C * K], FP32, kind="ExternalInput")
    out_d = nc.dram_tensor("out", [C, HW], FP32, kind="ExternalOutput")

    with tile.TileContext(nc) as tc, ExitStack() as ctx:
        pers = ctx.enter_context(tc.tile_pool(name="pers", bufs=1))
        dram = ctx.enter_context(tc.tile_pool(name="dram", bufs=1, space="DRAM"))

        # 4 row-bands: gathers for tiles 8g..8g+7 need only band g, so the
        # main loop starts after ~13 chunks instead of the full table.
        BANDS = [(0, 13), (1280, 12), (2432, 12), (3584, 13)]
        tables = []
        for _bi, (_, _nch) in enumerate(BANDS):
            tables.append(dram.tile([_nch * 128, 4 * C], BF16, name=f"tband{_bi}"))

        ident_f = pers.tile([128, 128], FP32)
        make_identity(nc, ident_f[:])
        ident_b = pers.tile([128, 128], BF16)
        nc.vector.tensor_copy(ident_b[:], ident_f[:])

        # per-partition constants: hh = p//64 (0/1), ww = p%64
        iota_p = pers.tile([128, 1], I32)
        nc.gpsimd.iota(iota_p[:], pattern=[[0, 1]], base=0, channel_multiplier=1)
        pf = pers.tile([128, 1], FP32)
        nc.vector.tensor_copy(pf[:], iota_p[:])
        hh = pers.tile([128, 1], FP32)
        nc.vector.tensor_scalar(out=hh[:], in0=pf[:], scalar1=64.0, scalar2=None,
                                op0=ALU.is_ge)
        ww = pers.tile([128, 1], FP32)
        nc.vector.scalar_tensor_tensor(out=ww[:], in0=hh[:], scalar=-64.0,
                                       in1=pf[:], op0=ALU.mult, op1=ALU.add)

        # batched base ramps over (t, k): by = 2t + ki + (PAD-1), bx = kj + (PAD-1)
        by_i = pers.tile([128, NPT, K], I32)
        nc.gpsimd.iota(by_i[:], pattern=[[2, NPT], [1, 3], [0, 3]], base=PAD - 1,
                       channel_multiplier=0)
        bx_i = pers.tile([128, NPT, K], I32)
        nc.gpsimd.iota(bx_i[:], pattern=[[0, NPT], [0, 3], [1, 3]], base=PAD - 1,
                       channel_multiplier=0)
        by_f = pers.tile([128, NPT, K], FP32)
        nc.vector.tensor_copy(by_f[:], by_i[:])
        bx_f = pers.tile([128, NPT, K], FP32)
        nc.vector.tensor_copy(bx_f[:], bx_i[:])

        w_defT = pers.tile([128, KT, 2 * 128], BF16)   # [ck-part, kt, o]
        r0_sb = pers.tile([128, NPT, K], I32)
        wts_sb = pers.tile([128, NPT, K * 3], FP32)    # k-major (rx, ry, rxry)
        r0T_f = pers.tile([108, 3], FP32)              # fixup staging
        r0T2_f = pers.tile([9, 36], FP32)              # r0T2_f[k, t]

        # ---------------- phases 1+3 scope (freed before main loop);
        # phase-2 pools stay open so the hi-band build overlaps the loop ----
        tblp = ctx.enter_context(tc.tile_pool(name="tblp", bufs=1))
        evb = ctx.enter_context(tc.tile_pool(name="evb", bufs=3))
        dyb = ctx.enter_context(tc.tile_pool(name="dyb", bufs=3))
        psB = ctx.enter_context(tc.tile_pool(name="psB", bufs=3, space="PSUM"))
        phx = ExitStack()
        xp = phx.enter_context(tc.tile_pool(name="xp", bufs=1))
        offp = phx.enter_context(tc.tile_pool(name="offp", bufs=1))
        scr = phx.enter_context(tc.tile_pool(name="scr", bufs=1))
        psW = phx.enter_context(tc.tile_pool(name="psW", bufs=1, space="PSUM"))
        psA = phx.enter_context(tc.tile_pool(name="psA", bufs=2, space="PSUM"))
        psT = phx.enter_context(tc.tile_pool(name="psT", bufs=2, space="PSUM"))

        # loads (x and w_adj cast to bf16 in-flight)
        x_sb = xp.tile([128, CT, HW], BF16)
        for ct in range(CT):
            nc.gpsimd.dma_start(out=x_sb[:, ct, :], in_=x_d[ct * 128:(ct + 1) * 128, :])
        w_adjT = offp.tile([128, CT, OFFC], BF16)
        for ct in range(CT):
            nc.gpsimd.dma_start(
                out=w_adjT[:, ct, :],
                in_=w_adj_d.rearrange("o c -> c o")[ct * 128:(ct + 1) * 128, :])
        b_adj_sb = offp.tile([OFFC, 1], FP32)
        nc.sync.dma_start(out=b_adj_sb[:], in_=b_adj_d[:, :])
        w_off_sb = offp.tile([OFFC, K], FP32)
        nc.sync.dma_start(out=w_off_sb[:], in_=w_off_d[:, :])
        b_off_sb = offp.tile([OFFC, 1], FP32)
        nc.sync.dma_start(out=b_off_sb[:], in_=b_off_d[:, :])

        # ---- phase 1: offsets pipeline ----
        GC = H + 2   # 66
        xch_pad = offp.tile([OFFC, GC * GC], BF16)
        nc.scalar.memzero(xch_pad[:])
        xch_v = xch_pad[:].rearrange("p (h w) -> p h w", h=GC, w=GC)
        for pch in range(8):
            ps = psA.tile([OFFC, 512], FP32)
            for ct in range(CT):
                nc.tensor.matmul(out=ps[:], lhsT=w_adjT[:, ct, :],
                                 rhs=x_sb[:, ct, pch * 512:(pch + 1) * 512],
                                 start=(ct == 0), stop=(ct == CT - 1))
            nc.scalar.activation(
                out=xch_v[:, 1 + pch * 8:1 + pch * 8 + 8, 1:1 + W],
                in_=ps[:].rearrange("p (h w) -> p h w", h=8, w=W),
                func=AF.Identity, bias=b_adj_sb[:], scale=1.0)

        # depthwise 3x3 on a 108-partition layout: partition = (row-block, ch),
        # 6 row-blocks of <=11 output rows, so each DVE op is [108, ~704]
        # instead of [18, 4096] (7x less time on the serial 9-tap chain).
        BLK = [(0, 11), (11, 11), (22, 11), (33, 11), (44, 11), (55, 9)]
        # scalar tiles replicated per block: w_off/b_off at partitions 18b+ch
        w_off6 = offp.tile([108, K], FP32)
        b_off6 = offp.tile([108, 1], FP32)
        for b in range(6):
            nc.sync.dma_start(out=w_off6[18 * b:18 * b + 18, :], in_=w_off_d[:, :])
            nc.sync.dma_start(out=b_off6[18 * b:18 * b + 18, :], in_=b_off_d[:, :])
        x2 = offp.tile([108, 13 * GC], BF16)
        nc.vector.memset(x2[:], 0.0)
        for b, (r0b, nr) in enumerate(BLK):
            nc.sync.dma_start(out=x2[18 * b:18 * b + 18, 0:(nr + 2) * GC],
                              in_=xch_pad[:, r0b * GC:(r0b + nr + 2) * GC])
        o2 = offp.tile([108, 11 * W], BF16)
        x2v = x2[:].rearrange("p (h w) -> p h w", h=13, w=GC)
        o2v = o2[:].rearrange("p (h w) -> p h w", h=11, w=W)
        nrmax = 11
        for tap in range(K):
            di, dj = tap // 3, tap % 3
            vin = x2v[:, di:di + nrmax, dj:dj + W]
            if tap == 0:
                nc.vector.tensor_scalar(
                    out=o2v, in0=vin, scalar1=w_off6[:, 0:1],
                    scalar2=b_off6[:, 0:1], op0=ALU.mult, op1=ALU.add)
            else:
                nc.vector.scalar_tensor_tensor(
                    out=o2v, in0=vin, scalar=w_off6[:, tap:tap + 1],
                    in1=o2v, op0=ALU.mult, op1=ALU.add)
        off_sb = offp.tile([OFFC, HW], BF16)
        for b, (r0b, nr) in enumerate(BLK):
            nc.sync.dma_start(out=off_sb[:, r0b * W:(r0b + nr) * W],
                              in_=o2[18 * b:18 * b + 18, 0:nr * W])

        offT = scr.tile([128, NPT, OFFC], FP32)
        for t in range(NPT):
            pso = psT.tile([128, OFFC], BF16, tag="pst")
            nc.tensor.transpose(pso[:], off_sb[:, t * 128:(t + 1) * 128],
                                ident_b[:OFFC, :OFFC])
            nc.scalar.copy(offT[:, t, :], pso[:])

        dyv = offT[:].rearrange("p t (k two) -> p t k two", two=2)[:, :, :, 0]
        dxv = offT[:].rearrange("p t (k two) -> p t k two", two=2)[:, :, :, 1]
        py = scr.tile([128, NPT, K], FP32)
        px = scr.tile([128, NPT, K], FP32)
        fyi = scr.tile([128, NPT, K], I32)
        fxi = scr.tile([128, NPT, K], I32)
        fy = scr.tile([128, NPT, K], FP32)
        fx = scr.tile([128, NPT, K], FP32)
        m = scr.tile([128, NPT, K], FP32)
        r0f = scr.tile([128, NPT, K], FP32)
        wv = wts_sb[:].rearrange("p t (k s) -> p t k s", s=3)
        # two passes: tiles 0-7 first so band-0 gathers can start early
        for (ta, tb) in [(0, 8), (8, NPT)]:
            sl = slice(ta, tb)
            nc.vector.scalar_tensor_tensor(out=py[:, sl], in0=dyv[:, sl],
                                           scalar=hh[:, 0:1], in1=by_f[:, sl],
                                           op0=ALU.add, op1=ALU.add)
            nc.vector.scalar_tensor_tensor(out=px[:, sl], in0=dxv[:, sl],
                                           scalar=ww[:, 0:1], in1=bx_f[:, sl],
                                           op0=ALU.add, op1=ALU.add)
            nc.vector.tensor_copy(fyi[:, sl], py[:, sl])
            nc.vector.tensor_copy(fxi[:, sl], px[:, sl])
            nc.vector.tensor_copy(fy[:, sl], fyi[:, sl])
            nc.vector.tensor_copy(fx[:, sl], fxi[:, sl])
            nc.vector.tensor_tensor(out=m[:, sl], in0=fy[:, sl], in1=py[:, sl],
                                    op=ALU.is_gt)
            nc.vector.tensor_sub(out=fy[:, sl], in0=fy[:, sl], in1=m[:, sl])
            nc.vector.tensor_tensor(out=m[:, sl], in0=fx[:, sl], in1=px[:, sl],
                                    op=ALU.is_gt)
            nc.vector.tensor_sub(out=fx[:, sl], in0=fx[:, sl], in1=m[:, sl])
            nc.vector.tensor_sub(out=wv[:, sl, :, 0], in0=px[:, sl], in1=fx[:, sl])
            nc.vector.tensor_sub(out=wv[:, sl, :, 1], in0=py[:, sl], in1=fy[:, sl])
            nc.vector.tensor_tensor(out=wv[:, sl, :, 2], in0=wv[:, sl, :, 0],
                                    in1=wv[:, sl, :, 1], op=ALU.mult)
            nc.vector.scalar_tensor_tensor(out=r0f[:, sl], in0=fy[:, sl],
                                           scalar=float(G), in1=fx[:, sl],
                                           op0=ALU.mult, op1=ALU.add)
            nc.vector.tensor_scalar(out=r0f[:, sl], in0=r0f[:, sl], scalar1=0.0,
                                    scalar2=float(RT - G - 2), op0=ALU.max,
                                    op1=ALU.min)
            if ta == 0:
                nc.vector.tensor_copy(r0_sb[:, 0:8, :], r0f[:, 0:8, :])
            else:
                for gb in range(1, 4):
                    nc.vector.tensor_scalar(
                        out=r0f[:, 8 * gb:8 * gb + 8, :],
                        in0=r0f[:, 8 * gb:8 * gb + 8, :],
                        scalar1=-float(BANDS[gb][0]), scalar2=None, op0=ALU.add)
                    nc.vector.tensor_copy(r0_sb[:, 8 * gb:8 * gb + 8, :],
                                          r0f[:, 8 * gb:8 * gb + 8, :])
        if GATHER == "multi":
            nc.vector.memset(r0T_f[:], 0.0)
            for j in range(3):
                nt = 12 if j < 2 else 8
                psr = psT.tile([108, 1], FP32, tag="psr")
                nc.tensor.transpose(psr[:9 * nt],
                                    r0f[0:1, 12 * j:12 * j + nt, :],
                                    ident_f[0:1, 0:1])
                nc.scalar.copy(r0T_f[:9 * nt, j:j + 1], psr[:9 * nt])
            r0T2_v = r0T2_f[:].rearrange("p (j w) -> p j w", w=12)
            for w in range(12):
                nj = 3 if w < 8 else 2
                nc.sync.dma_start(out=r0T2_v[:, 0:nj, w],
                                  in_=r0T_f[9 * w:9 * w + 9, 0:nj])

        w_def_sb = xp.tile([128, 2, C * K], BF16)      # cast-DMA fp32->bf16
        for ot in range(2):
            nc.gpsimd.dma_start(out=w_def_sb[:, ot, :],
                                in_=w_def_d[ot * 128:(ot + 1) * 128, :])
        # ---- phase 3: w_def transpose ----
        for kt in range(KT):
            k = kt // 2
            chalf = kt % 2
            for ot in range(2):
                ps = psW.tile([128, 128], BF16, tag="psw")
                src = w_def_sb[:, ot, :].rearrange("p (c k) -> p k c", k=K) \
                    [:, k, chalf * 128:(chalf + 1) * 128]
                nc.tensor.transpose(ps[:], src, ident_b[:])
                nc.scalar.copy(w_defT[:, kt, ot * 128:ot * 128 + 128], ps[:])

        # ---- phase 2: table build; Dy/Dxy per chunk ----
        xbf = tblp.tile([128, CT, RT], BF16)
        dxf = tblp.tile([128, CT, RT], BF16)
        head = PAD * G + PAD
        for ct in range(CT):
            # zero: head band, inter-row 8-col gaps, tail band
            nc.vector.memset(xbf[:, ct, 0:head], 0.0)
            nc.vector.memset(
                xbf[:, ct, head + W:head + W + (H - 1) * G]
                    .rearrange("p (r q) -> p r q", q=G)[:, :, 0:G - W], 0.0)
            nc.vector.memset(xbf[:, ct, head + (H - 1) * G + W:], 0.0)
            nc.vector.tensor_copy(
                xbf[:, ct, :ROWS].rearrange("p (h w) -> p h w", h=G, w=G)
                    [:, PAD:PAD + H, PAD:PAD + W],
                x_sb[:, ct, :].rearrange("p (h w) -> p h w", h=H, w=W))
        for ct in range(CT):
            nc.vector.tensor_sub(out=dxf[:, ct, 0:RT - 1],
                                 in0=xbf[:, ct, 1:RT], in1=xbf[:, ct, 0:RT - 1])
            nc.gpsimd.memset(dxf[:, ct, RT - 1:RT], 0.0)

        for band in range(4):
          btab = tables[band]
          for rt in range(BANDS[band][1]):
            lo = BANDS[band][0] + rt * 128
            n_y = min(128, RT - G - lo)    # rows with r+G in range
            tb = evb.tile([128, 4, C], BF16, tag="tb")
            for ct in range(CT):
                dyc = dyb.tile([128, 2, 128], BF16, tag="dy")
                e1 = nc.vector if band >= 1 else nc.gpsimd
                if n_y > 0:
                    e1.tensor_tensor(out=dyc[:, 0, :n_y],
                                     in0=xbf[:, ct, lo + G:lo + G + n_y],
                                     in1=xbf[:, ct, lo:lo + n_y],
                                     op=ALU.subtract)
                    e1.tensor_tensor(out=dyc[:, 1, :n_y],
                                     in0=dxf[:, ct, lo + G:lo + G + n_y],
                                     in1=dxf[:, ct, lo:lo + n_y],
                                     op=ALU.subtract)
                if n_y < 128:
                    nc.vector.memset(dyc[:, 0, max(n_y, 0):], 0.0)
                    nc.vector.memset(dyc[:, 1, max(n_y, 0):], 0.0)
                ps = psB.tile([128, 4 * 128], BF16, tag="ps")
                nc.tensor.transpose(ps[:, 0:128], xbf[:, ct, lo:lo + 128], ident_b[:])
                nc.tensor.transpose(ps[:, 128:256], dxf[:, ct, lo:lo + 128], ident_b[:])
                nc.tensor.transpose(ps[:, 256:384], dyc[:, 0, :], ident_b[:])
                nc.tensor.transpose(ps[:, 384:512], dyc[:, 1, :], ident_b[:])
                tbv = tb[:, :, ct * 128:(ct + 1) * 128]
                psv = ps[:].rearrange("p (s c) -> p s c", s=4)
                if (rt + ct) % 2 == 0:
                    nc.scalar.copy(tbv, psv)
                else:
                    nc.vector.tensor_copy(tbv, psv)
            nc.sync.dma_start(out=btab[rt * 128:(rt + 1) * 128, :], in_=tb[:])
        phx.close()

        # ---------------- main loop ----------------
        outp = ctx.enter_context(tc.tile_pool(name="outp", bufs=1))
        out_sb = outp.tile([128, 2, HW], FP32)
        with tc.tile_pool(name="gat", bufs=int(os.environ.get("GBUFS", "3"))) as gat, \
             tc.tile_pool(name="dia", bufs=2) as dia, \
             tc.tile_pool(name="fixp", bufs=2) as fixp, \
             tc.tile_pool(name="smp", bufs=2) as smp, \
             tc.tile_pool(name="psS", bufs=int(os.environ.get("PSBUFS", "3")), space="PSUM") as psS, \
             tc.tile_pool(name="psO", bufs=2, space="PSUM") as psO:
            for t in range(NPT):
                g_sb = gat.tile([128, K, 4 * C], BF16, tag="g")
                tabt = tables[t // 8]
                if GATHER == "multi":
                    nc.gpsimd.indirect_dma_start(
                        out=g_sb[:], out_offset=None,
                        in_=tabt[:, :],
                        in_offset=IndirectOffsetOnAxis(ap=r0_sb[:, t, :], axis=0),
                        bounds_check=BND - 1, oob_is_err=False)
                    idxf_f = fixp.tile([K, 1], FP32, tag="ixf")
                    nc.vector.scalar_tensor_tensor(
                        out=idxf_f[:], in0=g_sb[0:K, 0, 0:1], scalar=0.0,
                        in1=r0T2_f[:, t:t + 1],
                        op0=ALU.mult, op1=ALU.add)
                    idxf_i = fixp.tile([K, 1], I32, tag="ixi")
                    nc.vector.tensor_copy(idxf_i[:], idxf_f[:])
                    gF = fixp.tile([K, 4 * C], BF16, tag="gf")
                    nc.gpsimd.indirect_dma_start(
                        out=gF[:], out_offset=None,
                        in_=tabt[:, :],
                        in_offset=IndirectOffsetOnAxis(ap=idxf_i[:, :], axis=0))
                    nc.sync.dma_start(out=g_sb[0:1, :, :], in_=gF[:, :])
                else:
                    for k in range(K):
                        nc.gpsimd.indirect_dma_start(
                            out=g_sb[:, k, :], out_offset=None,
                            in_=tabt[:, :],
                            in_offset=IndirectOffsetOnAxis(ap=r0_sb[:, t, k:k + 1], axis=0))

                dg = dia.tile([128, K * 3, 128], BF16, tag="d")
                for k in range(K):
                    for s in range(3):
                        nc.vector.tensor_scalar(
                            out=dg[:, 3 * k + s, :], in0=ident_b[:],
                            scalar1=wts_sb[:, t, 3 * k + s:3 * k + s + 1],
                            scalar2=None, op0=ALU.mult)

                sampT = smp.tile([128, KT, 128], BF16, tag="st")
                for q in range(5):
                    n_in_g = 4 if q < 4 else 2
                    ps = psS.tile([128, 4 * 128], FP32, tag="pss")
                    for j in range(n_in_g):
                        kt = q * 4 + j
                        k = kt // 2
                        ch = kt % 2
                        win = ps[:, j * 128:(j + 1) * 128]
                        gv = g_sb[:, k, :].rearrange("p (s c) -> p s c", s=4)
                        nc.tensor.matmul(
                            out=win, lhsT=gv[:, 0, ch * 128:(ch + 1) * 128],
                            rhs=ident_b[:], start=True, stop=False)
                        for s in range(3):
                            nc.tensor.matmul(
                                out=win, lhsT=gv[:, s + 1, ch * 128:(ch + 1) * 128],
                                rhs=dg[:, 3 * k + s, :], start=False, stop=(s == 2))
                    nc.scalar.copy(sampT[:, q * 4:q * 4 + n_in_g, :],
                                   ps[:, :n_in_g * 128])

                for ot in range(2):
                    pso = psO.tile([128, 128], FP32, tag="po")
                    for kt in range(KT):
                        nc.tensor.matmul(out=pso[:],
                                         lhsT=w_defT[:, kt, ot * 128:(ot + 1) * 128],
                                         rhs=sampT[:, kt, :],
                                         start=(kt == 0), stop=(kt == KT - 1))
                    nc.scalar.copy(out_sb[:, ot, t * 128:(t + 1) * 128], pso[:])
                ogrp = [7, 15, 23, 27, 29, 30, 31]
                if t in ogrp:
                    t0o = ([-1] + ogrp)[ogrp.index(t)] + 1
                    for ot in range(2):
                        nc.sync.dma_start(
                            out=out_d[ot * 128:(ot + 1) * 128,
                                      t0o * 128:(t + 1) * 128],
                            in_=out_sb[:, ot, t0o * 128:(t + 1) * 128])
    return nc


_CACHE = {}


def _get_nc():
    if "nc" not in _CACHE:
        nc = build_nc()
        if not nc.is_finalized():
            nc.finalize()
        _CACHE["nc"] = nc
    return _CACHE["nc"]


def kernel(**inputs):
    from concourse import bass_utils
    x = np.ascontiguousarray(inputs["x"], dtype=np.float32)          # [8,256,64,64]
    w_adj = np.ascontiguousarray(inputs["w_adj"], dtype=np.float32).reshape(OFFC, C)
    b_adj = np.ascontiguousarray(inputs["b_adj"], dtype=np.float32).reshape(OFFC, 1)
    w_off = np.ascontiguousarray(inputs["w_off"], dtype=np.float32).reshape(OFFC, K)
    b_off = np.ascontiguousarray(inputs["b_off"], dtype=np.float32).reshape(OFFC, 1)
    w_def = np.ascontiguousarray(inputs["w_def"], dtype=np.float32).reshape(C, C * K)

    nc = _get_nc()
    in_maps = []
    for n in range(N):
        in_maps.append({
            "x": np.ascontiguousarray(x[n].reshape(C, HW)),
            "w_adj": w_adj, "b_adj": b_adj,
            "w_off": w_off, "b_off": b_off,
            "w_def": w_def,
        })
    res = bass_utils.run_bass_kernel_spmd(nc, in_maps, core_ids=list(range(N)))
    outs = [res.results[n]["out"].reshape(C, H, W) for n in range(N)]
    return np.stack(outs, axis=0)


if __name__ == "__main__":
    nc = build_nc()
    print("build ok")
